# revision 1
# baseline (speedup 1.0000x reference)
"""GroupMultiHeadAttention (GQA, causal, RoPE) Trainium2 Bass kernel.

Problem: x[1,2048,2048] -> MHA with H=32 heads, G=8 KV groups (4 heads/group),
head_dim=64, causal mask, RoPE on q/k, out proj. f32.

Sharding: 8-way tensor parallel by heads. Core c owns heads 4c..4c+3
(= KV group c): Wq/Wk/Wv column-sharded, Wo row-sharded. Each core produces
a partial y^T [D, L]; the host sums the 8 partials and transposes (this is
the gather/unshard step; no on-device collective needed).

Device-side layout strategy (per core, all matmuls in float32r):
  - x is PE-transposed on chip (128x128 tiles via identity matmul) into
    xT [d, l] blocks, 512 l-columns at a time.
  - Projections compute qT/kT/vT ([feat, l]) with d as the contraction
    (partition) dim, accumulating 16 k-tiles in PSUM.
  - RoPE: rotate_half is a 128x128 constant permutation matmul (PT), then
    q = q*cosT + rot(q)*sinT elementwise; cos/sin fed pre-transposed+stacked.
  - Scores are computed transposed: sT[kl, ql] = kT^T-free matmul with
    lhsT = kT tile, rhs = qT block. kT is duplicated into partitions 64..127
    so two heads run concurrently in disjoint PE row-groups.
  - Softmax without max-subtraction (logits are O(5), exp is safe in f32):
    exp on the scalar engine straight out of PSUM (scale=1/8 fused), causal
    mask applied only on diagonal tiles via a multiplicative tril block,
    and the denominator comes free from a ones-column appended to v
    (v_aug [kl, 65]); normalization divides by that row via a reciprocal +
    partition-broadcast DMA.
  - Out-projection consumes outT [feat, l] directly against a
    host-pre-transposed Wo slice, yielding yT [d, l] partials; causal
    structure skips all fully-masked kl tiles.
"""

import os
import numpy as np

import concourse.bass as bass
import concourse.tile as tile
from concourse import mybir
from concourse.masks import make_identity
from concourse.bass_utils import run_bass_kernel_spmd

F32R = mybir.dt.float32r
F32 = mybir.dt.float32

L = 2048          # sequence length
D = 2048          # model dim
HD = 64           # head dim
NHC = 4           # heads per core
FEAT = NHC * HD   # 256 per-core q features
LB = 512          # l block size
NLB = L // LB     # 4
KT = D // 128     # 16 contraction tiles
NCORES = 8


def _build_bass():
    nc = bass.Bass()

    x = nc.dram_tensor("x", [L, D], F32R, kind="ExternalInput")
    wqt = nc.dram_tensor("wqt", [D, FEAT], F32R, kind="ExternalInput")
    wkt = nc.dram_tensor("wkt", [D, HD], F32R, kind="ExternalInput")
    wvt = nc.dram_tensor("wvt", [D, HD], F32R, kind="ExternalInput")
    wot = nc.dram_tensor("wot", [FEAT, D], F32R, kind="ExternalInput")
    cost2 = nc.dram_tensor("cost2", [128, L], F32R, kind="ExternalInput")
    sint2 = nc.dram_tensor("sint2", [128, L], F32R, kind="ExternalInput")
    ptm = nc.dram_tensor("ptm", [128, 128], F32R, kind="ExternalInput")
    ltri = nc.dram_tensor("ltri", [LB, LB], F32R, kind="ExternalInput")
    onesc = nc.dram_tensor("onesc", [128, KT], F32R, kind="ExternalInput")
    iden = nc.dram_tensor("iden", [128, 128], F32R, kind="ExternalInput")
    yt = nc.dram_tensor("yt", [D, L], F32R, kind="ExternalOutput")

    with tile.TileContext(nc) as tc:
        with (
            tc.tile_pool(name="singles", bufs=1) as singles,
            tc.tile_pool(name="xsub", bufs=2) as xsub_p,
            tc.tile_pool(name="xt", bufs=1) as xt_p,
            tc.tile_pool(name="rtmp", bufs=3) as rtmp_p,
            tc.tile_pool(name="probs", bufs=3) as probs_p,
            tc.tile_pool(name="otmp", bufs=2) as otmp_p,
            tc.tile_pool(name="osum", bufs=2) as osum_p,
            tc.tile_pool(name="obc", bufs=2) as obc_p,
            tc.tile_pool(name="outsb", bufs=2) as outsb_p,
            tc.tile_pool(name="ytsb", bufs=3) as ytsb_p,
            tc.tile_pool(name="vtt", bufs=2) as vtt_p,
            tc.tile_pool(name="drb", bufs=2, space="DRAM") as drb_p,
            tc.tile_pool(name="ps_s", bufs=2, space="PSUM") as ps_s,
            tc.tile_pool(name="ps_o", bufs=2, space="PSUM") as ps_o,
            tc.tile_pool(name="ps_b", bufs=2, space="PSUM") as ps_b,
        ):
            # ---- resident tensors --------------------------------------
            # small tensors + identity first so PE can start transposing
            # the first x block while the big weight DMAs stream in.
            id_sb = singles.tile([128, 128], F32R)
            nc.sync.dma_start(id_sb, iden[:, :])
            pt_sb = singles.tile([128, 128], F32R)
            nc.sync.dma_start(pt_sb, ptm[:, :])
            cos_sb = singles.tile([128, L], F32R)
            nc.sync.dma_start(cos_sb, cost2[:, :])
            sin_sb = singles.tile([128, L], F32R)
            nc.sync.dma_start(sin_sb, sint2[:, :])
            ltri_sb = singles.tile([128, 4, LB], F32R)
            nc.sync.dma_start(ltri_sb, ltri.rearrange("(t p) q -> p t q", p=128))
            wkt_sb = singles.tile([128, KT, HD], F32R)
            nc.sync.dma_start(wkt_sb, wkt.rearrange("(k p) f -> p k f", p=128))
            wvt_sb = singles.tile([128, KT, HD], F32R)
            nc.sync.dma_start(wvt_sb, wvt.rearrange("(k p) f -> p k f", p=128))
            wqt_sb = singles.tile([128, KT, FEAT], F32R)
            wot_sb = singles.tile([128, 2, D], F32R)
            weights_loaded = [False]

            qt_sb = singles.tile([128, 2, L], F32R)     # roped qT, head pairs
            ktd_sb = singles.tile([128, L], F32R)       # roped kT, duplicated
            vaug_sb = singles.tile([128, KT, HD + 1], F32R)  # v with ones col
            nc.sync.dma_start(vaug_sb[:, :, HD:HD + 1],
                              onesc.rearrange("p (k o) -> p k o", o=1))

            copy_flip = [0]

            def copy_out(dst, src):
                # alternate PSUM->SBUF copies between DVE and ACT
                if copy_flip[0] % 2 == 0:
                    nc.vector.tensor_copy(dst, src)
                else:
                    nc.scalar.copy(dst, src)
                copy_flip[0] += 1

            for j in range(NLB):
                jsl = bass.ts(j, LB)
                # ---- load x rows for this l-block and transpose ---------
                xt_t = xt_p.tile([128, KT, LB], F32R, tag="xt")
                for sub in range(4):
                    xs = xsub_p.tile([128, D], F32R, tag="xsub")
                    nc.sync.dma_start(xs, x[j * LB + sub * 128:
                                            j * LB + (sub + 1) * 128, :])
                    ssl = slice(sub * 128, (sub + 1) * 128)
                    for kg in range(4):
                        tp = ps_s.tile([128, LB], F32R, tag="ps_s")
                        for kk in range(4):
                            nc.tensor.transpose(
                                tp[:, kk * 128:(kk + 1) * 128],
                                xs[:, (4 * kg + kk) * 128:
                                   (4 * kg + kk + 1) * 128],
                                id_sb,
                            )
                        copy_out(
                            xt_t[:, 4 * kg:4 * kg + 4, ssl],
                            tp.rearrange("p (k q) -> p k q", k=4),
                        )

                if not weights_loaded[0]:
                    nc.sync.dma_start(
                        wqt_sb, wqt.rearrange("(k p) f -> p k f", p=128))

                # ---- projections: qT (2 blocks), kT, vT -----------------
                def accumulate(lhs_of_k, m):
                    acc = ps_s.tile([128, LB], F32, tag="ps_s")
                    for k in range(KT):
                        nc.tensor.matmul(
                            acc[:m, :], lhs_of_k(k), xt_t[:, k, :],
                            start=(k == 0), stop=(k == KT - 1),
                        )
                    return acc

                def rope_into(dst, raw, rps):
                    # dst = raw * cos + rot(raw) * sin   (for this l block)
                    tmp = rtmp_p.tile([128, LB], F32R, tag="ropetmp")
                    nc.vector.tensor_mul(tmp, rps, sin_sb[:, jsl])
                    nc.vector.tensor_mul(dst, raw, cos_sb[:, jsl])
                    nc.vector.tensor_add(dst, dst, tmp)

                for fb in range(2):
                    acc = accumulate(
                        lambda k: wqt_sb[:, k, fb * 128:(fb + 1) * 128], 128)
                    raw = rtmp_p.tile([128, LB], F32R, tag="raw")
                    copy_out(raw, acc)
                    rps = ps_s.tile([128, LB], F32, tag="ps_s")
                    nc.tensor.matmul(rps, pt_sb, raw, start=True, stop=True)
                    rope_into(qt_sb[:, fb, jsl], raw, rps)

                acc = accumulate(lambda k: wkt_sb[:, k, :], HD)
                kraw = rtmp_p.tile([128, LB], F32R, tag="raw")
                nc.vector.tensor_copy(kraw[0:HD, :], acc[0:HD, :])
                nc.gpsimd.dma_start(kraw[HD:128, :], kraw[0:HD, :])  # dup rows
                rps = ps_s.tile([128, LB], F32, tag="ps_s")
                nc.tensor.matmul(rps, pt_sb, kraw, start=True, stop=True)
                rope_into(ktd_sb[:, jsl], kraw, rps)

                acc = accumulate(lambda k: wvt_sb[:, k, :], HD)
                vt_t = vtt_p.tile([HD, LB], F32R, tag="vtt")
                nc.vector.tensor_copy(vt_t, acc[0:HD, :])
                tp = ps_s.tile([128, LB], F32R, tag="ps_s")
                for i in range(4):
                    nc.tensor.transpose(
                        tp[:, i * HD:(i + 1) * HD],
                        vt_t[:, i * 128:(i + 1) * 128],
                        id_sb[0:HD, 0:HD],
                    )
                for i in range(4):
                    copy_out(vaug_sb[:, 4 * j + i, 0:HD],
                             tp[:, i * HD:(i + 1) * HD])

                # ---- attention for ql block j ---------------------------
                nkl = 4 * (j + 1)          # causal kl tiles
                out_t = outsb_p.tile([128, 2, LB], F32R, tag="outsb")
                for fb in range(2):
                    oA = ps_o.tile([HD + 1, LB], F32, tag="ps_o")
                    oB = ps_o.tile([HD + 1, LB], F32, tag="ps_o")
                    for pi in range(nkl // 2):
                        t0 = 2 * pi
                        sA = ps_b.tile([128, 2, LB], F32, tag="ps_b")
                        sB = ps_b.tile([128, 2, LB], F32, tag="ps_b")
                        for ti in range(2):
                            t = t0 + ti
                            ksl = bass.ts(t, 128)
                            nc.tensor.matmul(
                                sA[:, ti, :], ktd_sb[0:HD, ksl],
                                qt_sb[0:HD, fb, jsl],
                                start=True, stop=True)
                            nc.tensor.matmul(
                                sB[:, ti, :], ktd_sb[HD:128, ksl],
                                qt_sb[HD:128, fb, jsl],
                                start=True, stop=True)
                        pA = probs_p.tile([128, 2, LB], F32R, tag="probs")
                        pB = probs_p.tile([128, 2, LB], F32R, tag="probs")
                        nc.scalar.activation(
                            pA, sA, mybir.ActivationFunctionType.Exp,
                            scale=0.125)
                        nc.scalar.activation(
                            pB, sB, mybir.ActivationFunctionType.Exp,
                            scale=0.125)
                        for ti in range(2):
                            t = t0 + ti
                            if t >= 4 * j:   # diagonal tile: apply tril mask
                                i = t - 4 * j
                                nc.vector.tensor_mul(
                                    pA[:, ti, :], pA[:, ti, :],
                                    ltri_sb[:, i, :])
                                nc.vector.tensor_mul(
                                    pB[:, ti, :], pB[:, ti, :],
                                    ltri_sb[:, i, :])
                            nc.tensor.matmul(
                                oA, vaug_sb[:, t, :], pA[:, ti, :],
                                start=(t == 0), stop=(t == nkl - 1))
                            nc.tensor.matmul(
                                oB, vaug_sb[:, t, :], pB[:, ti, :],
                                start=(t == 0), stop=(t == nkl - 1))
                    # normalize: divide by the ones-row sums
                    for half, oX in ((0, oA), (1, oB)):
                        sums = osum_p.tile([HD + 1, LB], F32R, tag="osum")
                        with nc.allow_low_precision(reason="f32r is f32"):
                            nc.vector.reciprocal(sums[HD:HD + 1, :],
                                                 oX[HD:HD + 1, :])
                        bc = obc_p.tile([HD, LB], F32R, tag="obc")
                        db = drb_p.tile([1, LB], F32R, tag="drb")
                        nc.gpsimd.dma_start(db, sums[HD:HD + 1, :])
                        dsrc = db[0:1, :]
                        bc_src = bass.AP(
                            tensor=dsrc.tensor, offset=dsrc.offset,
                            ap=[[0, HD]] + [list(d) for d in dsrc.ap[1:]])
                        nc.gpsimd.dma_start(bc, bc_src)
                        if half == 0:
                            nc.vector.tensor_mul(
                                out_t[0:HD, fb, :], oX[0:HD, :], bc)
                        else:
                            ot = otmp_p.tile([HD, LB], F32R, tag="otmp")
                            nc.vector.tensor_mul(ot, oX[0:HD, :], bc)
                            nc.gpsimd.dma_start(out_t[HD:128, fb, :], ot)

                if not weights_loaded[0]:
                    nc.sync.dma_start(
                        wot_sb, wot.rearrange("(t p) d -> p t d", p=128))
                    weights_loaded[0] = True

                # ---- out projection for l block j -----------------------
                for dt in range(KT):
                    yp = ps_s.tile([128, LB], F32, tag="ps_s")
                    for kf in range(2):
                        nc.tensor.matmul(
                            yp, wot_sb[:, kf, dt * 128:(dt + 1) * 128],
                            out_t[:, kf, :],
                            start=(kf == 0), stop=(kf == 1))
                    ys = ytsb_p.tile([128, LB], F32R, tag="ytsb")
                    copy_out(ys, yp)
                    nc.sync.dma_start(yt[dt * 128:(dt + 1) * 128, jsl], ys)

    return nc


def _split_waits(nc, keep=1):
    """walrus in this container encodes at most one sync-wait per
    instruction; hoist extra waits into preceding same-engine NoOps."""
    for fn in nc.m.functions:
        for blk in fn.blocks:
            newl = []
            for ins in blk.instructions:
                si = ins.sync_info
                if (si is not None and si.on_wait is not None
                        and len(si.on_wait) > keep):
                    waits = list(si.on_wait)
                    extra, last = waits[:-keep], waits[-keep:]
                    for i, w in enumerate(extra):
                        nop = mybir.InstNoOp(name=f"{ins.name}-w{i}")
                        nop.engine = ins.engine
                        nop.sync_info = mybir.SyncInfo(on_wait=[w],
                                                       on_update=[])
                        newl.append(nop)
                    si.on_wait = last
                    ins.sync_info = si
                newl.append(ins)
            blk.instructions = newl


_NC_CACHE = None


def _get_nc():
    global _NC_CACHE
    if _NC_CACHE is None:
        _NC_CACHE = _build_bass()
        _split_waits(_NC_CACHE)
    return _NC_CACHE


def _host_prep(x, mask, cos, sin, Wq, Wk, Wv, Wo):
    """Build the 8 per-core input maps (sharding + layout transforms)."""
    x2d = np.ascontiguousarray(x.reshape(L, D).astype(np.float32))

    cosT = np.ascontiguousarray(cos.T.astype(np.float32))     # [64, L]
    sinT = np.ascontiguousarray(sin.T.astype(np.float32))
    cost2 = np.concatenate([cosT, cosT], axis=0)              # [128, L]
    sint2 = np.concatenate([sinT, sinT], axis=0)

    # rotate_half as a left-multiplication in [hd, l] layout:
    # rot(v) = P @ v with P[d, d+32] = -1 (d<32), P[d, d-32] = 1 (d>=32)
    P = np.zeros((HD, HD), dtype=np.float32)
    P[np.arange(32), np.arange(32) + 32] = -1.0
    P[np.arange(32, 64), np.arange(32, 64) - 32] = 1.0
    PT = P.T  # lhsT for the matmul
    ptm = np.zeros((128, 128), dtype=np.float32)
    ptm[0:64, 0:64] = PT
    ptm[64:128, 64:128] = PT
    ptm = np.ascontiguousarray(ptm)

    # multiplicative keep-mask, transposed, diagonal 512 block
    keepT = np.logical_not(np.asarray(mask)).T
    ltri = np.ascontiguousarray(keepT[:LB, :LB].astype(np.float32))

    in_maps = []
    for c in range(NCORES):
        fs = slice(c * FEAT, (c + 1) * FEAT)
        gs = slice(c * HD, (c + 1) * HD)
        in_maps.append({
            "x": x2d,
            "wqt": np.ascontiguousarray(Wq[fs, :].T.astype(np.float32)),
            "wkt": np.ascontiguousarray(Wk[gs, :].T.astype(np.float32)),
            "wvt": np.ascontiguousarray(Wv[gs, :].T.astype(np.float32)),
            "wot": np.ascontiguousarray(Wo[:, fs].T.astype(np.float32)),
            "cost2": cost2,
            "sint2": sint2,
            "ptm": ptm,
            "ltri": ltri,
            "onesc": np.ones((128, KT), dtype=np.float32),
            "iden": np.eye(128, dtype=np.float32),
        })
    return in_maps


def _combine(results):
    acc = results[0]["yt"].astype(np.float32)
    for r in results[1:]:
        acc = acc + r["yt"]
    return np.ascontiguousarray(acc.T)[None, :, :].astype(np.float32)


def kernel(**inputs):
    nc = _get_nc()
    in_maps = _host_prep(**inputs)
    res = run_bass_kernel_spmd(nc, in_maps, list(range(NCORES)))
    return _combine(res.results)


def kernel_profiled(**inputs):
    """Like kernel() but returns (output, exec_time_ns, raw BassKernelResults)."""
    nc = _get_nc()
    in_maps = _host_prep(**inputs)
    res = run_bass_kernel_spmd(nc, in_maps, list(range(NCORES)), trace=True)
    return _combine(res.results), res.exec_time_ns, res

